# revision 22
# baseline (speedup 1.0000x reference)
"""Trainium2 Bass kernel for nn_BinaryDilGroupConv.

Reference computation (B=32, C=256, H=W=56, GROUPS=4):
    c1  = conv2d(sign(x), sign(w1), stride=2, pad=1, groups=4)   # -> (B,256,28,28)
    x1  = batchnorm_train(c1, g1, b1) + maxpool3x3s2p1(x)
    c2  = conv2d(sign(x1), sign(w2), 1x1)
    out = batchnorm_train(c2, g2, b2) + x1

Strategy: data-parallel over batch across 8 NeuronCores (4 images/core),
f16 input/output (host casts; rel-err budget 2e-2 >> f16 noise).

Key device-side restructuring vs a direct translation:
  * sign(x) is replaced by q = (x >= 0) in {0,1} (DVE is_ge at 4x rate);
    conv weights are 2*sign(w); the missing "-1" contribution is a
    per-channel constant absorbed by batchnorm, except at the top/left
    output edge of conv1 where the zero-padding makes it per-pixel -- a
    rank-3 correction matmul (masks precomputed on host) fixes that
    exactly inside PSUM.  Same trick for conv2 (1x1: fully absorbed).
  * Input x is stored column-deinterleaved ([row][parity][col28]) so the
    stride-2 conv rhs reads and all four maxpool ops are unit-stride
    (DVE 16-bit 2x mode); maxpool horizontal stage runs on GpSimd for
    half the tiles to offload the DVE.
  * BN batch stats are taken as sum/sumsq via the Scalar engine's
    activation accumulate port: the PSUM->SBUF evict (Copy) yields the
    sum for free, one extra Square pass yields sumsq; no DVE bn_stats.
  * BN affine is fused: x1 = (c1*s1)+m in one DVE scalar_tensor_tensor,
    q2 = (x1 >= -b1v) in one DVE tensor_scalar, and the output is
    (c2*s2 + (b1v+b2v)) + x1 in two DVE ops.  The per-channel BN biases
    ride the is_ge threshold / final affine so no separate bias pass.
  * Global stats use one tiny [128,4] f32 AllReduce per stage; a dummy
    warmup AllReduce issued at kernel start absorbs the collective
    path's cold-start / launch-skew cost concurrently with phase A.
"""

import sys

for _p in ("/opt/trn_rl_repo", "/root/.axon_site/_ro/trn_rl_repo"):
    if _p not in sys.path:
        sys.path.append(_p)

import numpy as np

import concourse.bass as bass
import concourse.bacc as bacc
import concourse.mybir as mybir
import concourse.tile as tile
from concourse import bass_utils, library_config

# Use a direct SBUF-to-SBUF remote-DMA exchange for the FIRST tiny
# BN-stat all-reduce (a one-shot XOR-relative broadcast allgather +
# local reduce, ~5us).  The second all-reduce stays on the ncfw
# collective path: SWDGE ring reuse for a second remote-DMA exchange
# stalls for milliseconds (hw-observed), while the ncfw path's one-time
# ~55us init barrier self-runs concurrently with phases A-C and only
# its ~12us execution lands on the critical path.  RDMA=False falls
# back to ncfw for both.
RDMA = True
RDMA_AR2 = False

N_CORES = 8
B, C, H, W = 32, 256, 56, 56
BL = B // N_CORES          # images per core
OH = OW = 28
NPIX = OH * OW             # 784
NGLB = B * NPIX            # samples/channel for global stats (25088)
EPS = 1e-5

F32 = mybir.dt.float32
F16 = mybir.dt.float16
BF16 = mybir.dt.bfloat16

AF = mybir.ActivationFunctionType
ALU = mybir.AluOpType

RG = [list(range(N_CORES))]

# conv1 kernel positions, center (1,1) last so it carries stop=True
KPOS = [(0, 0), (0, 1), (0, 2), (1, 0), (1, 2), (2, 0), (2, 1), (2, 2), (1, 1)]


def _emit_conv1_pair(nc, ps_list, sx_list, w1t, ec, masks, h):
    """conv1 for a pair of images (shared weight loads), half h.

    sx tiles are [128, 56, 2, 28] (row, parity, col) {0,1}-valued f16;
    weights are 2*sign(w).  psum layout: rows 0-13 at bank0 cols 0-391,
    rows 14-27 at bank1 (offset 512).
    """
    # edge correction first: writes every pixel -> start=True
    for ps in ps_list:
        for b in range(2):
            nc.tensor.matmul(
                ps[:, 512 * b:512 * b + 392],
                ec[:, 128 * h:128 * h + 128],
                masks[:, 392 * b:392 * b + 392],
                start=True, stop=False)
    for kh, kw in KPOS:
        i0 = 1 if kh == 0 else 0
        j0 = 1 if kw == 0 else 0
        ncol = 28 - j0
        parity = 0 if kw == 1 else 1
        coff = 0  # odd-col index offset is 0 for both kw=0 and kw=2
        woff = ((kh * 3 + kw) * 2 + h) * 128
        last = (kh == 1 and kw == 1)
        for ps, sx in zip(ps_list, sx_list):
            for b in range(2):
                r0 = max(i0, 14 * b)
                nr = 14 * b + 14 - r0
                a0 = 2 * r0 + kh - 1
                bank = ps[:, 512 * b:512 * b + 392].rearrange(
                    "p (r c) -> p r c", c=28)
                out_v = bank[:, r0 - 14 * b:r0 - 14 * b + nr, j0:28]
                rhs = sx[:, a0:a0 + 2 * (nr - 1) + 1:2, parity,
                         coff:coff + ncol]
                nc.tensor.matmul(
                    out_v, w1t[:, woff:woff + 128], rhs,
                    start=False, stop=last)


def _affine_from_sq(nc, vp, gpk, g_ap, b_ap, s_out, b_out, thr_out, tag):
    """BN affine from packed global [128, 2(h), 2(sum,sumsq)] moments.

    s_out = g/sqrt(var+eps); b_out = b - mean*s; thr_out = -b_out.
    """
    mg = vp.tile([128, 2], F32, name="mg", tag=f"mg{tag}")
    nc.vector.tensor_scalar(mg[:], gpk[:, :, 0], 1.0 / NGLB, None, ALU.mult)
    e2 = vp.tile([128, 2], F32, name="e2", tag=f"e2{tag}")
    nc.vector.tensor_scalar(e2[:], gpk[:, :, 1], 1.0 / NGLB, None, ALU.mult)
    vr = vp.tile([128, 2], F32, name="vr", tag=f"vr{tag}")
    nc.vector.tensor_mul(vr[:], mg[:], mg[:])
    nc.vector.tensor_sub(vr[:], e2[:], vr[:])
    nc.vector.tensor_scalar(vr[:], vr[:], EPS, None, ALU.add)
    sd = vp.tile([128, 2], F32, name="sd", tag=f"sd{tag}")
    nc.scalar.sqrt(sd[:], vr[:])
    inv = vp.tile([128, 2], F32, name="inv", tag=f"inv{tag}")
    nc.vector.reciprocal(inv[:], sd[:])
    nc.vector.tensor_mul(s_out, inv[:], g_ap)
    t2 = vp.tile([128, 2], F32, name="t2", tag=f"t2{tag}")
    nc.vector.tensor_mul(t2[:], mg[:], s_out)
    nc.vector.tensor_sub(b_out, b_ap, t2[:])
    nc.vector.tensor_sub(thr_out, t2[:], b_ap)


def _inject_waits(nc, injections):
    """Append sem-ge waits to named instructions after Tile scheduling.

    Tile's single-core scheduling sim cannot model semaphore increments
    arriving from peer cores' remote DMAs, so those waits are attached
    to the already-scheduled program (the carrying instructions are in
    the local data-dependency chain, which pins their engine order).
    """
    by_name = {}
    for fn in nc.m.functions:
        for blk in fn.blocks:
            for ins in blk.instructions:
                by_name[ins.name] = ins
    for name, sem, val in injections:
        ins = by_name[name]
        w = mybir.SyncWait(
            sync_type="semaphore", id=sem.num, ant_name=sem.name,
            wait_mode="sem-ge-imm", wait_value=val)
        if ins.sync_info is None:
            ins.sync_info = mybir.SyncInfo(on_wait=[w], on_update=[])
        else:
            ins.sync_info.on_wait.append(w)


def _build():
    nc = bacc.Bacc(
        "TRN2",
        target_bir_lowering=False,
        debug=False,
        enable_asserts=False,
        num_devices=N_CORES,
    )
    xs = nc.dram_tensor("xs", [BL, C, H * W], F16, kind="ExternalInput")
    w1b = nc.dram_tensor("w1b", [128, 2304], F16, kind="ExternalInput")
    w2b = nc.dram_tensor("w2b", [128, 512], F16, kind="ExternalInput")
    ecb = nc.dram_tensor("ecb", [3, 256], F16, kind="ExternalInput")
    mkb = nc.dram_tensor("mkb", [3, 784], F16, kind="ExternalInput")
    vecs_d = nc.dram_tensor("vecs", [128, 8], F32, kind="ExternalInput")
    out_d = nc.dram_tensor("out", [BL, C, NPIX], F16, kind="ExternalOutput")

    xs_ap = xs.ap()
    out_ap = out_d.ap()

    if RDMA:
        rsem = nc.alloc_semaphore("ar_rsem")
        lsem = nc.alloc_semaphore("ar_lsem")
        psem = nc.alloc_semaphore("ar_psem")
        injections = []

    with tile.TileContext(nc) as tc:
        with tc.tile_pool(name="wp", bufs=1) as wp, \
             tc.tile_pool(name="xp", bufs=4) as xp, \
             tc.tile_pool(name="sxp", bufs=3) as sxp, \
             tc.tile_pool(name="tp", bufs=8) as tp, \
             tc.tile_pool(name="sqp", bufs=2) as sqp, \
             tc.tile_pool(name="vp", bufs=2) as vp, \
             tc.tile_pool(name="pp", bufs=4, space="PSUM") as pp, \
             tc.tile_pool(name="dramp", bufs=1, space="DRAM") as dramp:

            wmt = wp.tile([128, 1], F32)
            nc.vector.memset(wmt[:], 0.0)

            if RDMA:
                nc.gpsimd.load_library(library_config.remote_dma)
                send1 = wp.tile([128, 4], F32)
                gbuf1 = wp.tile([128, 8, 4], F32)
                send2 = wp.tile([128, 4], F32)
                gbuf2 = wp.tile([128, 8, 4], F32)

                def emit_preps(gbuf, send, gate_lsem=None):
                    for j in range(1, 8):
                        # D2D lanes (bit2) deliver to dest^2 (hw lane perm)
                        dj = j ^ 2 if j & 4 else j
                        rd = [(0, dj) if k == j else None for k in range(8)]
                        bi = nc.gpsimd.remote_dma_broadcast(
                            gbuf[:, j], send[:], rsem, lsem, rdests=rd)
                        if gate_lsem and j == 1:
                            # don't desc-gen while the ring still holds the
                            # previous exchange's frames
                            injections.append((bi.ins.name, lsem, gate_lsem))

                emit_preps(gbuf1, send1)
            else:
                # ---- warmup collective: absorbs ncfw cold-start / skew
                # while phase A streams.  Keep its trigger path minimal.
                with tc.high_priority():
                    warm_in = dramp.tile([128, 1], F32)
                    warm_out = dramp.tile([128, 1], F32)
                    nc.sync.dma_start(warm_in[:], wmt[:])
                    nc.gpsimd.collective_compute(
                        "AllReduce", ALU.add, replica_groups=RG,
                        ins=[warm_in.opt()], outs=[warm_out.opt()])

            # ---- constants ----
            w1t = wp.tile([128, 2304], F16)
            nc.scalar.dma_start(w1t[:], w1b.ap())
            w2t = wp.tile([128, 512], F16)
            nc.scalar.dma_start(w2t[:], w2b.ap())
            ec = wp.tile([3, 256], F16)
            nc.scalar.dma_start(ec[:], ecb.ap())
            masks = wp.tile([3, 784], F16)
            nc.scalar.dma_start(masks[:], mkb.ap())
            vecs = wp.tile([128, 8], F32)
            nc.scalar.dma_start(vecs[:], vecs_d.ap())

            # warm the ACT table sets (sqrt is on the AR critical path)
            wsq = wp.tile([128, 1], F32)
            nc.scalar.sqrt(wsq[:], wmt[:])
            nc.scalar.square(wsq[:], wmt[:])

            # persistent big tiles
            c1t = wp.tile([128, 2, BL, NPIX], F16)     # conv1 (h, n)
            mbt = wp.tile([128, 2, BL, NPIX], F16)     # maxpool (h, n)
            x1t = wp.tile([128, 2, BL, NPIX], F16)     # x1 = s1*c1 + m
            sx1t = wp.tile([128, 2, BL, NPIX], F16)    # q2 (h, n)
            c2t = wp.tile([128, 2, BL, NPIX], F16)     # conv2 (co, n)
            otb = wp.tile([128, 2, BL, NPIX], F16)     # output (co, n)
            st1 = wp.tile([128, 2, 2, BL], F32)        # [h, kind, n]
            st2 = wp.tile([128, 2, 2, BL], F32)        # [co, kind, n]
            pk1 = wp.tile([128, 2, 2], F32)
            pk2 = wp.tile([128, 2, 2], F32)
            g1pk = wp.tile([128, 2, 2], F32)
            g2pk = wp.tile([128, 2, 2], F32)
            s1v = wp.tile([128, 2], F32)
            b1v = wp.tile([128, 2], F32)
            thr1 = wp.tile([128, 2], F32)
            s2v = wp.tile([128, 2], F32)
            b2v = wp.tile([128, 2], F32)
            thr2d = wp.tile([128, 2], F32)
            bbv = wp.tile([128, 2], F32)

            sx_t = {}

            # ======== phase A: load, q1, conv1, moments; maxpool deferred
            def stage_a_load(h, n):
                xt = xp.tile([128, H * W], F16, name="xt")
                nc.sync.dma_start(xt[:], xs_ap[n, 128 * h:128 * h + 128])
                sx = sxp.tile([128, H, 2, OW], F16, name="sx")
                nc.vector.tensor_scalar(
                    sx.rearrange("p a b c -> p (a b c)"), xt[:],
                    0.0, None, ALU.is_ge)
                sx_t[(h, n)] = (xt, sx)

            def stage_a_stats(h, n, ps):
                pv = ps.rearrange("p (b x) -> p b x", x=512)[:, :, 0:392]
                cv = c1t[:, h, n].rearrange("p (b x) -> p b x", b=2)
                nc.scalar.activation(
                    cv, pv, AF.Copy, accum_out=st1[:, h, 0, n:n + 1])
                sq = sqp.tile([128, 2, 392], BF16, name="sq")
                nc.scalar.activation(
                    sq[:], pv, AF.Square, accum_out=st1[:, h, 1, n:n + 1])

            t_t = {}

            def stage_a_vpool(h, n):
                # vertical maxpool stage: the last reader of xt
                xt, _ = sx_t[(h, n)]
                xv = xt.rearrange("p (a b c) -> p a b c", a=H, b=2)
                t = tp.tile([128, OH, 2, OW], F16, name="t")
                nc.vector.tensor_max(t[:], xv[:, 0:56:2], xv[:, 1:56:2])
                nc.vector.tensor_max(t[:, 1:28], t[:, 1:28], xv[:, 1:54:2])
                t_t[(h, n)] = t

            def stage_a_hpool(h, n):
                # horizontal stage, deferred into the AllReduce window
                t = t_t[(h, n)]
                m = mbt[:, h, n].rearrange("p (a b) -> p a b", a=OH)
                nc.vector.tensor_max(m[:, :, :], t[:, :, 0], t[:, :, 1])
                nc.vector.tensor_max(m[:, :, 1:28], m[:, :, 1:28],
                                     t[:, :, 1, 0:27])

            def _pack(st, dest, base):
                for a in range(2):
                    for k in range(2):
                        u = vp.tile([128, 2], F32, name="u",
                                    tag=f"u{base}{a}{k}")
                        nc.vector.tensor_add(
                            u[:], st[:, a, k, 0:2], st[:, a, k, 2:4])
                        nc.vector.tensor_add(
                            dest[:, 2 * a + k:2 * a + k + 1],
                            u[:, 0:1], u[:, 1:2])

            def _reduce_gather(gbuf, send, gpk, rtarget, tag):
                cp0 = nc.vector.tensor_copy(gbuf[:, 0], send[:])
                injections.append((cp0.ins.name, rsem, rtarget))
                r1 = vp.tile([128, 4, 4], F32, name="r1", tag=f"r1{tag}")
                nc.vector.tensor_add(r1[:], gbuf[:, 0:4], gbuf[:, 4:8])
                r2 = vp.tile([128, 2, 4], F32, name="r2", tag=f"r2{tag}")
                nc.vector.tensor_add(r2[:], r1[:, 0:2], r1[:, 2:4])
                nc.vector.tensor_add(
                    gpk.rearrange("p a b -> p (a b)"), r2[:, 0], r2[:, 1])

            def ar1_send():
                with tc.high_priority():
                    _pack(st1, send1, "s1")
                    nc.vector.sem_inc(psem, 1)
                    nc.gpsimd.wait_ge(psem, 1)
                    nc.gpsimd.trigger_dma(count=7)
                if RDMA_AR2:
                    # AR2's desc-gen follows on the gpsimd FIFO (runs
                    # during phase C), gated on AR1's ring retirement
                    emit_preps(gbuf2, send2, gate_lsem=112)

            def ar1_fin():
                _reduce_gather(gbuf1, send1, g1pk, 14, "g1")
                _affine_from_sq(
                    nc, vp, g1pk, vecs[:, 0:2], vecs[:, 2:4],
                    s1v[:], b1v[:], thr1[:], tag="a1")

            def ar1():
              with tc.high_priority():
                _pack(st1, pk1.rearrange("p a b -> p (a b)"), "s1")
                ain = dramp.tile([128, 4], F32, name="ar1in")
                aout = dramp.tile([128, 4], F32, name="ar1out")
                nc.sync.dma_start(ain[:], pk1.rearrange("p a b -> p (a b)"))
                nc.gpsimd.collective_compute(
                    "AllReduce", ALU.add, replica_groups=RG,
                    ins=[ain.opt()], outs=[aout.opt()])
                nc.sync.dma_start(g1pk.rearrange("p a b -> p (a b)"), aout[:])
                _affine_from_sq(
                    nc, vp, g1pk, vecs[:, 0:2], vecs[:, 2:4],
                    s1v[:], b1v[:], thr1[:], tag="a1")

            for h in range(2):
                for n in range(BL):
                    stage_a_load(h, n)
                for w in range(2):
                    ns = (2 * w, 2 * w + 1)
                    ps_list = [pp.tile([128, 1024], F32, name="ps",
                                       tag="ps") for _ in ns]
                    _emit_conv1_pair(
                        nc, ps_list,
                        [sx_t[(h, n)][1] for n in ns], w1t, ec, masks, h)
                    for ps, n in zip(ps_list, ns):
                        stage_a_stats(h, n, ps)
                        stage_a_vpool(h, n)
            if RDMA:
                ar1_send()
            else:
                ar1()
            # horizontal maxpool overlaps the exchange latency
            for h in range(2):
                for n in range(BL):
                    stage_a_hpool(h, n)
            if RDMA:
                ar1_fin()

            # ======== phase C: x1, q2, conv2, moments ========
            def stage_c1(h, n):
                mfl = mbt[:, h, n]
                nc.vector.scalar_tensor_tensor(
                    x1t[:, h, n], c1t[:, h, n], s1v[:, h:h + 1], mfl,
                    ALU.mult, ALU.add)
                nc.vector.tensor_scalar(
                    sx1t[:, h, n], x1t[:, h, n], thr1[:, h:h + 1], None,
                    ALU.is_ge)

            def stage_c2(co, n):
                ps2 = pp.tile([128, 1024], F32, name="ps2", tag="ps")
                for ci in range(2):
                    woff = (ci * 2 + co) * 128
                    for cc0, ccn in ((0, 512), (512, NPIX - 512)):
                        nc.tensor.matmul(
                            ps2[:, cc0:cc0 + ccn],
                            w2t[:, woff:woff + 128],
                            sx1t[:, ci, n][:, cc0:cc0 + ccn],
                            start=(ci == 0), stop=(ci == 1))
                cv = c2t[:, co, n]
                nc.scalar.activation(
                    cv, ps2[:, 0:NPIX], AF.Copy,
                    accum_out=st2[:, co, 0, n:n + 1])
                sq = sqp.tile([128, 2, 392], BF16, name="sq")
                nc.scalar.activation(
                    sq.rearrange("p a b -> p (a b)"), ps2[:, 0:NPIX],
                    AF.Square, accum_out=st2[:, co, 1, n:n + 1])

            def ar2():
                if RDMA and RDMA_AR2:
                    _pack(st2, send2, "s2")
                    nc.gpsimd.wait_ge(psem, 8)
                    nc.gpsimd.trigger_dma(count=7)
                    _reduce_gather(gbuf2, send2, g2pk, 28, "g2")
                    _affine_from_sq(
                        nc, vp, g2pk, vecs[:, 4:6], vecs[:, 6:8],
                        s2v[:], b2v[:], thr2d[:], tag="a2")
                    nc.vector.tensor_add(bbv[:], b1v[:], b2v[:])
                    return
                with tc.high_priority():
                    _pack(st2, pk2.rearrange("p a b -> p (a b)"), "s2")
                    ain = dramp.tile([128, 4], F32, name="ar2in")
                    aout = dramp.tile([128, 4], F32, name="ar2out")
                    nc.sync.dma_start(
                        ain[:], pk2.rearrange("p a b -> p (a b)"))
                    nc.gpsimd.collective_compute(
                        "AllReduce", ALU.add, replica_groups=RG,
                        ins=[ain.opt()], outs=[aout.opt()])
                    nc.sync.dma_start(
                        g2pk.rearrange("p a b -> p (a b)"), aout[:])
                    _affine_from_sq(
                        nc, vp, g2pk, vecs[:, 4:6], vecs[:, 6:8],
                        s2v[:], b2v[:], thr2d[:], tag="a2")
                    nc.vector.tensor_add(bbv[:], b1v[:], b2v[:])

            for n in range(BL):
                stage_c1(0, n)
                stage_c1(1, n)
            for co in range(2):
                for n in range(BL):
                    stage_c2(co, n)
            ar2()

            # ======== phase E: out = (s2*c2 + b1v+b2v) + x1 ========
            for co in range(2):
                for n in range(BL):
                    nc.vector.tensor_scalar(
                        otb[:, co, n], c2t[:, co, n],
                        s2v[:, co:co + 1], bbv[:, co:co + 1],
                        ALU.mult, ALU.add)
                    nc.vector.tensor_add(
                        otb[:, co, n], otb[:, co, n], x1t[:, co, n])
                nc.sync.dma_start(
                    out_ap[:, 128 * co:128 * co + 128].rearrange(
                        "n p x -> p n x"),
                    otb[:, co])

    if RDMA:
        _inject_waits(nc, injections)
    nc.compile()
    return nc


_NC = None


def _get_nc():
    global _NC
    if _NC is None:
        _NC = _build()
    return _NC


def _prep_inputs(x, w1, g1, b1, w2, g2, b2):
    """Host-side dtype/layout prep (weights tiny; x cast+deinterleave)."""
    x = np.asarray(x, dtype=np.float32)
    # column de-interleave: [B, C, H, 28, 2] -> [B, C, H, 2, 28]
    xr = x.reshape(B, C, H, OW, 2).transpose(0, 1, 2, 4, 3)
    x16 = np.ascontiguousarray(xr.reshape(B, C, H * W)).astype(np.float16)

    sw1 = np.sign(w1.astype(np.float32))            # [256, 64, 3, 3]
    t1 = np.zeros((128, 3, 3, 2, 128), np.float32)  # [ci_l, kh, kw, h, co_l]
    for h in range(2):
        for bb in range(2):
            blk = sw1[128 * h + 64 * bb:128 * h + 64 * bb + 64]
            t1[64 * bb:64 * bb + 64, :, :, h, 64 * bb:64 * bb + 64] = \
                2.0 * blk.transpose(1, 2, 3, 0)
    w1bv = t1.reshape(128, 2304).astype(np.float16)

    colsum1 = sw1.sum(axis=1)                       # [256, 3, 3]
    ecw = np.stack([
        colsum1[:, 0, :].sum(-1),                   # rowsum_top
        colsum1[:, :, 0].sum(-1),                   # colsum_left
        -colsum1[:, 0, 0],                          # -c00
    ]).astype(np.float16)                           # [3, 256]

    mk = np.zeros((3, OH, OW), np.float32)
    mk[0, 0, :] = 1.0
    mk[1, :, 0] = 1.0
    mk[2, 0, 0] = 1.0
    mkv = mk.reshape(3, NPIX).astype(np.float16)

    sw2 = np.sign(w2.astype(np.float32)[:, :, 0, 0])  # [256 co, 256 ci]
    t2 = np.zeros((128, 2, 2, 128), np.float32)       # [ci_l, ci, co, co_l]
    for ci in range(2):
        for co in range(2):
            t2[:, ci, co, :] = 2.0 * sw2[128 * co:128 * co + 128,
                                         128 * ci:128 * ci + 128].T
    w2bv = t2.reshape(128, 512).astype(np.float16)

    vecs = np.zeros((128, 8), np.float32)
    vecs[:, 0] = g1[:128]
    vecs[:, 1] = g1[128:]
    vecs[:, 2] = b1[:128]
    vecs[:, 3] = b1[128:]
    vecs[:, 4] = g2[:128]
    vecs[:, 5] = g2[128:]
    vecs[:, 6] = b2[:128]
    vecs[:, 7] = b2[128:]

    in_maps = []
    for i in range(N_CORES):
        in_maps.append({
            "xs": np.ascontiguousarray(x16[BL * i:BL * (i + 1)]),
            "w1b": w1bv,
            "w2b": w2bv,
            "ecb": ecw,
            "mkb": mkv,
            "vecs": vecs,
        })
    return in_maps


def run(x, w1, g1, b1, w2, g2, b2, trace=False):
    nc = _get_nc()
    in_maps = _prep_inputs(x, w1, g1, b1, w2, g2, b2)
    res = bass_utils.run_bass_kernel_spmd(
        nc, in_maps, core_ids=list(range(N_CORES)), trace=trace)
    out = np.concatenate(
        [res.results[i]["out"] for i in range(N_CORES)], axis=0)
    out = out.reshape(B, C, OH, OW).astype(np.float32)
    return out, res


def kernel(**inputs):
    out, _ = run(
        inputs["x"], inputs["w1"], inputs["g1"], inputs["b1"],
        inputs["w2"], inputs["g2"], inputs["b2"])
    return out


# revision 28
# speedup vs baseline: 1.0566x; 1.0566x over previous
"""Trainium2 Bass kernel for nn_BinaryDilGroupConv.

Reference computation (B=32, C=256, H=W=56, GROUPS=4):
    c1  = conv2d(sign(x), sign(w1), stride=2, pad=1, groups=4)   # -> (B,256,28,28)
    x1  = batchnorm_train(c1, g1, b1) + maxpool3x3s2p1(x)
    c2  = conv2d(sign(x1), sign(w2), 1x1)
    out = batchnorm_train(c2, g2, b2) + x1

Strategy: data-parallel over batch across 8 NeuronCores (4 images/core),
f16 input/output (host casts; rel-err budget 2e-2 >> f16 noise).

Key device-side restructuring vs a direct translation:
  * sign(x) is replaced by q = (x >= 0) in {0,1} (DVE is_ge at 4x rate);
    conv weights are 2*sign(w); the missing "-1" contribution is a
    per-channel constant absorbed by batchnorm, except at the top/left
    output edge of conv1 where the zero-padding makes it per-pixel -- a
    rank-3 correction matmul (masks precomputed on host) fixes that
    exactly inside PSUM.  Same trick for conv2 (1x1: fully absorbed).
  * Input x is stored column-deinterleaved ([row][parity][col28]) so the
    stride-2 conv rhs reads and all four maxpool ops are unit-stride
    (DVE 16-bit 2x mode); maxpool horizontal stage runs on GpSimd for
    half the tiles to offload the DVE.
  * BN batch stats are taken as sum/sumsq via the Scalar engine's
    activation accumulate port: the PSUM->SBUF evict (Copy) yields the
    sum for free, one extra Square pass yields sumsq; no DVE bn_stats.
  * BN affine is fused: x1 = (c1*s1)+m in one DVE scalar_tensor_tensor,
    q2 = (x1 >= -b1v) in one DVE tensor_scalar, and the output is
    (c2*s2 + (b1v+b2v)) + x1 in two DVE ops.  The per-channel BN biases
    ride the is_ge threshold / final affine so no separate bias pass.
  * Global stats use one tiny [128,4] f32 AllReduce per stage; a dummy
    warmup AllReduce issued at kernel start absorbs the collective
    path's cold-start / launch-skew cost concurrently with phase A.
"""

import sys

for _p in ("/opt/trn_rl_repo", "/root/.axon_site/_ro/trn_rl_repo"):
    if _p not in sys.path:
        sys.path.append(_p)

import numpy as np

import concourse.bass as bass
import concourse.bacc as bacc
import concourse.mybir as mybir
import concourse.tile as tile
from concourse import bass_utils, library_config

# Use a direct SBUF-to-SBUF remote-DMA exchange for the FIRST tiny
# BN-stat all-reduce (a one-shot XOR-relative broadcast allgather +
# local reduce, ~5us).  The second all-reduce stays on the ncfw
# collective path: SWDGE ring reuse for a second remote-DMA exchange
# stalls for milliseconds (hw-observed), while the ncfw path's one-time
# ~55us init barrier self-runs concurrently with phases A-C and only
# its ~12us execution lands on the critical path.  RDMA=False falls
# back to ncfw for both.
RDMA = True
RDMA_AR2 = False

N_CORES = 8
B, C, H, W = 32, 256, 56, 56
BL = B // N_CORES          # images per core
OH = OW = 28
NPIX = OH * OW             # 784
NGLB = B * NPIX            # samples/channel for global stats (25088)
EPS = 1e-5

F32 = mybir.dt.float32
F16 = mybir.dt.float16
BF16 = mybir.dt.bfloat16

AF = mybir.ActivationFunctionType
ALU = mybir.AluOpType

RG = [list(range(N_CORES))]

# conv1 kernel positions, center (1,1) last so it carries stop=True
KPOS = [(0, 0), (0, 1), (0, 2), (1, 0), (1, 2), (2, 0), (2, 1), (2, 2), (1, 1)]


def _emit_conv1_pair(nc, ps_list, sx_list, w1t, ec, masks, h):
    """conv1 for a pair of images (shared weight loads), half h.

    sx tiles are [128, 56, 2, 28] (row, parity, col) {0,1}-valued f16;
    weights are 2*sign(w).  psum layout: rows 0-13 at bank0 cols 0-391,
    rows 14-27 at bank1 (offset 512).
    """
    # edge correction first: writes every pixel -> start=True
    for ps in ps_list:
        for b in range(2):
            nc.tensor.matmul(
                ps[:, 512 * b:512 * b + 392],
                ec[:, 128 * h:128 * h + 128],
                masks[:, 392 * b:392 * b + 392],
                start=True, stop=False)
    for kh, kw in KPOS:
        i0 = 1 if kh == 0 else 0
        j0 = 1 if kw == 0 else 0
        ncol = 28 - j0
        parity = 0 if kw == 1 else 1
        coff = 0  # odd-col index offset is 0 for both kw=0 and kw=2
        woff = ((kh * 3 + kw) * 2 + h) * 128
        last = (kh == 1 and kw == 1)
        for ps, sx in zip(ps_list, sx_list):
            for b in range(2):
                r0 = max(i0, 14 * b)
                nr = 14 * b + 14 - r0
                a0 = 2 * r0 + kh - 1
                bank = ps[:, 512 * b:512 * b + 392].rearrange(
                    "p (r c) -> p r c", c=28)
                out_v = bank[:, r0 - 14 * b:r0 - 14 * b + nr, j0:28]
                rhs = sx[:, a0:a0 + 2 * (nr - 1) + 1:2, parity,
                         coff:coff + ncol]
                nc.tensor.matmul(
                    out_v, w1t[:, woff:woff + 128], rhs,
                    start=False, stop=last)


def _affine_from_sq(nc, vp, gpk, g_ap, b_ap, s_out, b_out, thr_out, tag):
    """BN affine from packed global [128, 2(h), 2(sum,sumsq)] moments.

    s_out = g/sqrt(var+eps); b_out = b - mean*s; thr_out = -b_out.
    """
    mg = vp.tile([128, 2], F32, name="mg", tag=f"mg{tag}")
    nc.vector.tensor_scalar(mg[:], gpk[:, :, 0], 1.0 / NGLB, None, ALU.mult)
    e2 = vp.tile([128, 2], F32, name="e2", tag=f"e2{tag}")
    nc.vector.tensor_scalar(e2[:], gpk[:, :, 1], 1.0 / NGLB, None, ALU.mult)
    vr = vp.tile([128, 2], F32, name="vr", tag=f"vr{tag}")
    nc.vector.tensor_mul(vr[:], mg[:], mg[:])
    nc.vector.tensor_sub(vr[:], e2[:], vr[:])
    nc.vector.tensor_scalar(vr[:], vr[:], EPS, None, ALU.add)
    sd = vp.tile([128, 2], F32, name="sd", tag=f"sd{tag}")
    nc.scalar.sqrt(sd[:], vr[:])
    inv = vp.tile([128, 2], F32, name="inv", tag=f"inv{tag}")
    nc.vector.reciprocal(inv[:], sd[:])
    nc.vector.tensor_mul(s_out, inv[:], g_ap)
    t2 = vp.tile([128, 2], F32, name="t2", tag=f"t2{tag}")
    nc.vector.tensor_mul(t2[:], mg[:], s_out)
    nc.vector.tensor_sub(b_out, b_ap, t2[:])
    nc.vector.tensor_sub(thr_out, t2[:], b_ap)


def _inject_waits(nc, injections):
    """Append sem-ge waits to named instructions after Tile scheduling.

    Tile's single-core scheduling sim cannot model semaphore increments
    arriving from peer cores' remote DMAs, so those waits are attached
    to the already-scheduled program (the carrying instructions are in
    the local data-dependency chain, which pins their engine order).
    """
    by_name = {}
    for fn in nc.m.functions:
        for blk in fn.blocks:
            for ins in blk.instructions:
                by_name[ins.name] = ins
    for name, sem, val in injections:
        ins = by_name[name]
        w = mybir.SyncWait(
            sync_type="semaphore", id=sem.num, ant_name=sem.name,
            wait_mode="sem-ge-imm", wait_value=val)
        if ins.sync_info is None:
            ins.sync_info = mybir.SyncInfo(on_wait=[w], on_update=[])
        else:
            ins.sync_info.on_wait.append(w)


def _build():
    nc = bacc.Bacc(
        "TRN2",
        target_bir_lowering=False,
        debug=False,
        enable_asserts=False,
        num_devices=N_CORES,
    )
    xs = nc.dram_tensor("xs", [BL, C, H * W], F16, kind="ExternalInput")
    w1b = nc.dram_tensor("w1b", [128, 2304], F16, kind="ExternalInput")
    w2b = nc.dram_tensor("w2b", [128, 512], F16, kind="ExternalInput")
    ecb = nc.dram_tensor("ecb", [3, 256], F16, kind="ExternalInput")
    mkb = nc.dram_tensor("mkb", [3, 784], F16, kind="ExternalInput")
    vecs_d = nc.dram_tensor("vecs", [128, 8], F32, kind="ExternalInput")
    out_d = nc.dram_tensor("out", [BL, C, NPIX], F16, kind="ExternalOutput")

    xs_ap = xs.ap()
    out_ap = out_d.ap()

    if RDMA:
        rsem = nc.alloc_semaphore("ar_rsem")
        lsem = nc.alloc_semaphore("ar_lsem")
        psem = nc.alloc_semaphore("ar_psem")
        injections = []

    with tile.TileContext(nc) as tc:
        with tc.tile_pool(name="wp", bufs=1) as wp, \
             tc.tile_pool(name="xp", bufs=4) as xp, \
             tc.tile_pool(name="sxp", bufs=4) as sxp, \
             tc.tile_pool(name="tp", bufs=8) as tp, \
             tc.tile_pool(name="sqp", bufs=2) as sqp, \
             tc.tile_pool(name="vp", bufs=2) as vp, \
             tc.tile_pool(name="pp", bufs=4, space="PSUM") as pp, \
             tc.tile_pool(name="dramp", bufs=1, space="DRAM") as dramp:

            wmt = wp.tile([128, 1], F32)
            nc.vector.memset(wmt[:], 0.0)

            # ---- warmup collective: the ncfw path's one-time init barrier
            # and first-op cost run concurrently with phases A-C so the real
            # stage-2 AllReduce executes warm.
            with tc.high_priority():
                warm_in = dramp.tile([128, 1], F32)
                warm_out = dramp.tile([128, 1], F32)
                nc.sync.dma_start(warm_in[:], wmt[:])
                nc.gpsimd.collective_compute(
                    "AllReduce", ALU.add, replica_groups=RG,
                    ins=[warm_in.opt()], outs=[warm_out.opt()])

            if RDMA:
                nc.gpsimd.load_library(library_config.remote_dma)
                send1 = wp.tile([128, 4], F32)
                gbuf1 = wp.tile([128, 8, 4], F32)
                send2 = wp.tile([128, 4], F32)
                gbuf2 = wp.tile([128, 8, 4], F32)

                def emit_preps(gbuf, send, gate_lsem=None):
                    for j in range(1, 8):
                        # D2D lanes (bit2) deliver to dest^2 (hw lane perm)
                        dj = j ^ 2 if j & 4 else j
                        rd = [(0, dj) if k == j else None for k in range(8)]
                        bi = nc.gpsimd.remote_dma_broadcast(
                            gbuf[:, j], send[:], rsem, lsem, rdests=rd)
                        if gate_lsem and j == 1:
                            # don't desc-gen while the ring still holds the
                            # previous exchange's frames
                            injections.append((bi.ins.name, lsem, gate_lsem))

                emit_preps(gbuf1, send1)


            # ---- constants ----
            w1t = wp.tile([128, 2304], F16)
            nc.scalar.dma_start(w1t[:], w1b.ap())
            w2t = wp.tile([128, 512], F16)
            nc.scalar.dma_start(w2t[:], w2b.ap())
            ec = wp.tile([3, 256], F16)
            nc.scalar.dma_start(ec[:], ecb.ap())
            masks = wp.tile([3, 784], F16)
            nc.scalar.dma_start(masks[:], mkb.ap())
            vecs = wp.tile([128, 8], F32)
            nc.scalar.dma_start(vecs[:], vecs_d.ap())

            # warm the ACT table sets (sqrt is on the AR critical path)
            wsq = wp.tile([128, 1], F32)
            nc.scalar.sqrt(wsq[:], wmt[:])
            nc.scalar.square(wsq[:], wmt[:])

            # persistent big tiles
            c1t = wp.tile([128, 2, BL, NPIX], F16)     # conv1 (h, n)
            mbt = wp.tile([128, 2, BL, NPIX], F16)     # maxpool (h, n)
            x1t = wp.tile([128, 2, BL, NPIX], F16)     # x1 = s1*c1 + m
            sx1t = wp.tile([128, 2, BL, NPIX], F16)    # q2 (h, n)
            c2t = wp.tile([128, 2, BL, NPIX], F16)     # conv2 (co, n)
            otb = wp.tile([128, 2, BL, NPIX], F16)     # output (co, n)
            st1 = wp.tile([128, 2, 2, BL], F32)        # [h, kind, n]
            st2 = wp.tile([128, 2, 2, BL], F32)        # [co, kind, n]
            pk1 = wp.tile([128, 2, 2], F32)
            pk2 = wp.tile([128, 2, 2], F32)
            g1pk = wp.tile([128, 2, 2], F32)
            g2pk = wp.tile([128, 2, 2], F32)
            s1v = wp.tile([128, 2], F32)
            b1v = wp.tile([128, 2], F32)
            thr1 = wp.tile([128, 2], F32)
            s2v = wp.tile([128, 2], F32)
            b2v = wp.tile([128, 2], F32)
            thr2d = wp.tile([128, 2], F32)
            bbv = wp.tile([128, 2], F32)

            sx_t = {}

            # ======== phase A: load, q1, conv1, moments; maxpool deferred
            def stage_a_load(h, n):
                xt = xp.tile([128, H * W], F16, name="xt")
                nc.sync.dma_start(xt[:], xs_ap[n, 128 * h:128 * h + 128])
                sx = sxp.tile([128, H, 2, OW], F16, name="sx")
                nc.vector.tensor_scalar(
                    sx.rearrange("p a b c -> p (a b c)"), xt[:],
                    0.0, None, ALU.is_ge)
                sx_t[(h, n)] = (xt, sx)

            def stage_a_stats(h, n, ps):
                pv = ps.rearrange("p (b x) -> p b x", x=512)[:, :, 0:392]
                cv = c1t[:, h, n].rearrange("p (b x) -> p b x", b=2)
                nc.scalar.activation(
                    cv, pv, AF.Copy, accum_out=st1[:, h, 0, n:n + 1])
                sq = sqp.tile([128, 2, 392], BF16, name="sq")
                nc.scalar.activation(
                    sq[:], pv, AF.Square, accum_out=st1[:, h, 1, n:n + 1])

            t_t = {}

            def stage_a_vpool(h, n):
                # vertical maxpool stage: the last reader of xt
                xt, _ = sx_t[(h, n)]
                xv = xt.rearrange("p (a b c) -> p a b c", a=H, b=2)
                t = tp.tile([128, OH, 2, OW], F16, name="t")
                nc.vector.tensor_max(t[:], xv[:, 0:56:2], xv[:, 1:56:2])
                nc.vector.tensor_max(t[:, 1:28], t[:, 1:28], xv[:, 1:54:2])
                t_t[(h, n)] = t

            def stage_a_hpool(h, n):
                # horizontal stage, deferred into the AllReduce window
                t = t_t[(h, n)]
                m = mbt[:, h, n].rearrange("p (a b) -> p a b", a=OH)
                nc.vector.tensor_max(m[:, :, :], t[:, :, 0], t[:, :, 1])
                nc.vector.tensor_max(m[:, :, 1:28], m[:, :, 1:28],
                                     t[:, :, 1, 0:27])

            def _pack(st, dest, base):
                for a in range(2):
                    for k in range(2):
                        u = vp.tile([128, 2], F32, name="u",
                                    tag=f"u{base}{a}{k}")
                        nc.vector.tensor_add(
                            u[:], st[:, a, k, 0:2], st[:, a, k, 2:4])
                        nc.vector.tensor_add(
                            dest[:, 2 * a + k:2 * a + k + 1],
                            u[:, 0:1], u[:, 1:2])

            def _reduce_gather(gbuf, send, gpk, rtarget, tag):
                cp0 = nc.vector.tensor_copy(gbuf[:, 0], send[:])
                r1 = vp.tile([128, 4, 4], F32, name="r1", tag=f"r1{tag}")
                nc.vector.tensor_add(r1[:], gbuf[:, 0:4], gbuf[:, 4:8])
                r2 = vp.tile([128, 2, 4], F32, name="r2", tag=f"r2{tag}")
                nc.vector.tensor_add(r2[:], r1[:, 0:2], r1[:, 2:4])
                nc.vector.tensor_add(
                    gpk.rearrange("p a b -> p (a b)"), r2[:, 0], r2[:, 1])

            def ar1_send():
                with tc.high_priority():
                    _pack(st1, send1, "s1")
                    nc.vector.sem_inc(psem, 1)
                    nc.gpsimd.wait_ge(psem, 1)
                    nc.gpsimd.trigger_dma(count=7)
                if RDMA_AR2:
                    # AR2's desc-gen follows on the gpsimd FIFO (runs
                    # during phase C), gated on AR1's ring retirement
                    emit_preps(gbuf2, send2, gate_lsem=112)

            def ar1_fin():
                # marker op reading the last hpool output: pins the DVE
                # queue so the blocking rsem wait sits AFTER the maxpool
                # work (which fills the exchange-latency window)
                mk = nc.vector.tensor_copy(
                    gbuf1[:, 0, 0:1], mbt[:, 1, BL - 1, 0:1])
                injections.append((mk.ins.name, rsem, 14))
                _reduce_gather(gbuf1, send1, g1pk, 14, "g1")
                _affine_from_sq(
                    nc, vp, g1pk, vecs[:, 0:2], vecs[:, 2:4],
                    s1v[:], b1v[:], thr1[:], tag="a1")

            def ar1():
              with tc.high_priority():
                _pack(st1, pk1.rearrange("p a b -> p (a b)"), "s1")
                ain = dramp.tile([128, 4], F32, name="ar1in")
                aout = dramp.tile([128, 4], F32, name="ar1out")
                nc.sync.dma_start(ain[:], pk1.rearrange("p a b -> p (a b)"))
                nc.gpsimd.collective_compute(
                    "AllReduce", ALU.add, replica_groups=RG,
                    ins=[ain.opt()], outs=[aout.opt()])
                nc.sync.dma_start(g1pk.rearrange("p a b -> p (a b)"), aout[:])
                _affine_from_sq(
                    nc, vp, g1pk, vecs[:, 0:2], vecs[:, 2:4],
                    s1v[:], b1v[:], thr1[:], tag="a1")

            for h in range(2):
                for n in range(BL):
                    stage_a_load(h, n)
                for w in range(2):
                    ns = (2 * w, 2 * w + 1)
                    ps_list = [pp.tile([128, 1024], F32, name="ps",
                                       tag="ps") for _ in ns]
                    _emit_conv1_pair(
                        nc, ps_list,
                        [sx_t[(h, n)][1] for n in ns], w1t, ec, masks, h)
                    for ps, n in zip(ps_list, ns):
                        stage_a_stats(h, n, ps)
                        stage_a_vpool(h, n)
            if RDMA:
                ar1_send()
            else:
                ar1()
            # horizontal maxpool overlaps the exchange latency
            for h in range(2):
                for n in range(BL):
                    stage_a_hpool(h, n)
            if RDMA:
                ar1_fin()

            # ======== phase C: x1, q2, conv2, moments ========
            def stage_c1(h, n):
                mfl = mbt[:, h, n]
                nc.vector.scalar_tensor_tensor(
                    x1t[:, h, n], c1t[:, h, n], s1v[:, h:h + 1], mfl,
                    ALU.mult, ALU.add)
                nc.vector.tensor_scalar(
                    sx1t[:, h, n], x1t[:, h, n], thr1[:, h:h + 1], None,
                    ALU.is_ge)

            def stage_c2(co, n):
                ps2 = pp.tile([128, 1024], F32, name="ps2", tag="ps")
                for ci in range(2):
                    woff = (ci * 2 + co) * 128
                    for cc0, ccn in ((0, 512), (512, NPIX - 512)):
                        nc.tensor.matmul(
                            ps2[:, cc0:cc0 + ccn],
                            w2t[:, woff:woff + 128],
                            sx1t[:, ci, n][:, cc0:cc0 + ccn],
                            start=(ci == 0), stop=(ci == 1))
                cv = c2t[:, co, n]
                nc.scalar.activation(
                    cv, ps2[:, 0:NPIX], AF.Copy,
                    accum_out=st2[:, co, 0, n:n + 1])
                sq = sqp.tile([128, 2, 392], BF16, name="sq")
                nc.scalar.activation(
                    sq.rearrange("p a b -> p (a b)"), ps2[:, 0:NPIX],
                    AF.Square, accum_out=st2[:, co, 1, n:n + 1])

            def ar2():
                if RDMA and RDMA_AR2:
                    _pack(st2, send2, "s2")
                    nc.vector.sem_inc(psem, 1)
                    nc.gpsimd.wait_ge(psem, 2)
                    nc.gpsimd.trigger_dma(count=7)
                    mk2 = nc.vector.tensor_copy(
                        gbuf2[:, 0, 0:1], send2[:, 0:1])
                    injections.append((mk2.ins.name, rsem, 28))
                    _reduce_gather(gbuf2, send2, g2pk, 28, "g2")
                    _affine_from_sq(
                        nc, vp, g2pk, vecs[:, 4:6], vecs[:, 6:8],
                        s2v[:], b2v[:], thr2d[:], tag="a2")
                    nc.vector.tensor_add(bbv[:], b1v[:], b2v[:])
                    return
                with tc.high_priority():
                    _pack(st2, pk2.rearrange("p a b -> p (a b)"), "s2")
                    ain = dramp.tile([128, 4], F32, name="ar2in")
                    aout = dramp.tile([128, 4], F32, name="ar2out")
                    nc.sync.dma_start(
                        ain[:], pk2.rearrange("p a b -> p (a b)"))
                    nc.gpsimd.collective_compute(
                        "AllReduce", ALU.add, replica_groups=RG,
                        ins=[ain.opt()], outs=[aout.opt()])
                    nc.sync.dma_start(
                        g2pk.rearrange("p a b -> p (a b)"), aout[:])
                    _affine_from_sq(
                        nc, vp, g2pk, vecs[:, 4:6], vecs[:, 6:8],
                        s2v[:], b2v[:], thr2d[:], tag="a2")
                    nc.vector.tensor_add(bbv[:], b1v[:], b2v[:])

            for n in range(BL):
                stage_c1(0, n)
                stage_c1(1, n)
            for co in range(2):
                for n in range(BL):
                    stage_c2(co, n)
            ar2()

            # ======== phase E: out = (s2*c2 + b1v+b2v) + x1 ========
            for co in range(2):
                for n in range(BL):
                    nc.vector.tensor_scalar(
                        otb[:, co, n], c2t[:, co, n],
                        s2v[:, co:co + 1], bbv[:, co:co + 1],
                        ALU.mult, ALU.add)
                    nc.vector.tensor_add(
                        otb[:, co, n], otb[:, co, n], x1t[:, co, n])
                nc.sync.dma_start(
                    out_ap[:, 128 * co:128 * co + 128].rearrange(
                        "n p x -> p n x"),
                    otb[:, co])

    if RDMA:
        _inject_waits(nc, injections)
    nc.compile()
    return nc


_NC = None


def _get_nc():
    global _NC
    if _NC is None:
        _NC = _build()
    return _NC


def _prep_inputs(x, w1, g1, b1, w2, g2, b2):
    """Host-side dtype/layout prep (weights tiny; x cast+deinterleave)."""
    x = np.asarray(x, dtype=np.float32)
    # column de-interleave: [B, C, H, 28, 2] -> [B, C, H, 2, 28]
    xr = x.reshape(B, C, H, OW, 2).transpose(0, 1, 2, 4, 3)
    x16 = np.ascontiguousarray(xr.reshape(B, C, H * W)).astype(np.float16)

    sw1 = np.sign(w1.astype(np.float32))            # [256, 64, 3, 3]
    t1 = np.zeros((128, 3, 3, 2, 128), np.float32)  # [ci_l, kh, kw, h, co_l]
    for h in range(2):
        for bb in range(2):
            blk = sw1[128 * h + 64 * bb:128 * h + 64 * bb + 64]
            t1[64 * bb:64 * bb + 64, :, :, h, 64 * bb:64 * bb + 64] = \
                2.0 * blk.transpose(1, 2, 3, 0)
    w1bv = t1.reshape(128, 2304).astype(np.float16)

    colsum1 = sw1.sum(axis=1)                       # [256, 3, 3]
    ecw = np.stack([
        colsum1[:, 0, :].sum(-1),                   # rowsum_top
        colsum1[:, :, 0].sum(-1),                   # colsum_left
        -colsum1[:, 0, 0],                          # -c00
    ]).astype(np.float16)                           # [3, 256]

    mk = np.zeros((3, OH, OW), np.float32)
    mk[0, 0, :] = 1.0
    mk[1, :, 0] = 1.0
    mk[2, 0, 0] = 1.0
    mkv = mk.reshape(3, NPIX).astype(np.float16)

    sw2 = np.sign(w2.astype(np.float32)[:, :, 0, 0])  # [256 co, 256 ci]
    t2 = np.zeros((128, 2, 2, 128), np.float32)       # [ci_l, ci, co, co_l]
    for ci in range(2):
        for co in range(2):
            t2[:, ci, co, :] = 2.0 * sw2[128 * co:128 * co + 128,
                                         128 * ci:128 * ci + 128].T
    w2bv = t2.reshape(128, 512).astype(np.float16)

    vecs = np.zeros((128, 8), np.float32)
    vecs[:, 0] = g1[:128]
    vecs[:, 1] = g1[128:]
    vecs[:, 2] = b1[:128]
    vecs[:, 3] = b1[128:]
    vecs[:, 4] = g2[:128]
    vecs[:, 5] = g2[128:]
    vecs[:, 6] = b2[:128]
    vecs[:, 7] = b2[128:]

    in_maps = []
    for i in range(N_CORES):
        in_maps.append({
            "xs": np.ascontiguousarray(x16[BL * i:BL * (i + 1)]),
            "w1b": w1bv,
            "w2b": w2bv,
            "ecb": ecw,
            "mkb": mkv,
            "vecs": vecs,
        })
    return in_maps


def run(x, w1, g1, b1, w2, g2, b2, trace=False):
    nc = _get_nc()
    in_maps = _prep_inputs(x, w1, g1, b1, w2, g2, b2)
    res = bass_utils.run_bass_kernel_spmd(
        nc, in_maps, core_ids=list(range(N_CORES)), trace=trace)
    out = np.concatenate(
        [res.results[i]["out"] for i in range(N_CORES)], axis=0)
    out = out.reshape(B, C, OH, OW).astype(np.float32)
    return out, res


def kernel(**inputs):
    out, _ = run(
        inputs["x"], inputs["w1"], inputs["g1"], inputs["b1"],
        inputs["w2"], inputs["g2"], inputs["b2"])
    return out


# revision 30
# speedup vs baseline: 1.1186x; 1.0587x over previous
"""Trainium2 Bass kernel for nn_BinaryDilGroupConv.

Reference computation (B=32, C=256, H=W=56, GROUPS=4):
    c1  = conv2d(sign(x), sign(w1), stride=2, pad=1, groups=4)   # -> (B,256,28,28)
    x1  = batchnorm_train(c1, g1, b1) + maxpool3x3s2p1(x)
    c2  = conv2d(sign(x1), sign(w2), 1x1)
    out = batchnorm_train(c2, g2, b2) + x1

Strategy: data-parallel over batch across 8 NeuronCores (4 images/core),
f16 input/output (host casts; rel-err budget 2e-2 >> f16 noise).

Key device-side restructuring vs a direct translation:
  * sign(x) is replaced by q = (x >= 0) in {0,1} (DVE is_ge at 4x rate);
    conv weights are 2*sign(w); the missing "-1" contribution is a
    per-channel constant absorbed by batchnorm, except at the top/left
    output edge of conv1 where the zero-padding makes it per-pixel -- a
    rank-3 correction matmul (masks precomputed on host) fixes that
    exactly inside PSUM.  Same trick for conv2 (1x1: fully absorbed).
  * Input x is stored column-deinterleaved ([row][parity][col28]) so the
    stride-2 conv rhs reads and all four maxpool ops are unit-stride
    (DVE 16-bit 2x mode); maxpool horizontal stage runs on GpSimd for
    half the tiles to offload the DVE.
  * BN batch stats are taken as sum/sumsq via the Scalar engine's
    activation accumulate port: the PSUM->SBUF evict (Copy) yields the
    sum for free, one extra Square pass yields sumsq; no DVE bn_stats.
  * BN affine is fused: x1 = (c1*s1)+m in one DVE scalar_tensor_tensor,
    q2 = (x1 >= -b1v) in one DVE tensor_scalar, and the output is
    (c2*s2 + (b1v+b2v)) + x1 in two DVE ops.  The per-channel BN biases
    ride the is_ge threshold / final affine so no separate bias pass.
  * Global stats use one tiny [128,4] f32 AllReduce per stage; a dummy
    warmup AllReduce issued at kernel start absorbs the collective
    path's cold-start / launch-skew cost concurrently with phase A.
"""

import sys

for _p in ("/opt/trn_rl_repo", "/root/.axon_site/_ro/trn_rl_repo"):
    if _p not in sys.path:
        sys.path.append(_p)

import numpy as np

import concourse.bass as bass
import concourse.bacc as bacc
import concourse.mybir as mybir
import concourse.tile as tile
from concourse import bass_utils, library_config

# Use a direct SBUF-to-SBUF remote-DMA exchange for the FIRST tiny
# BN-stat all-reduce (a one-shot XOR-relative broadcast allgather +
# local reduce, ~5us).  The second all-reduce stays on the ncfw
# collective path: SWDGE ring reuse for a second remote-DMA exchange
# stalls for milliseconds (hw-observed), while the ncfw path's one-time
# ~55us init barrier self-runs concurrently with phases A-C and only
# its ~12us execution lands on the critical path.  RDMA=False falls
# back to ncfw for both.
RDMA = True
RDMA_AR2 = False

N_CORES = 8
B, C, H, W = 32, 256, 56, 56
BL = B // N_CORES          # images per core
OH = OW = 28
NPIX = OH * OW             # 784
NGLB = B * NPIX            # samples/channel for global stats (25088)
EPS = 1e-5

F32 = mybir.dt.float32
F16 = mybir.dt.float16
BF16 = mybir.dt.bfloat16

AF = mybir.ActivationFunctionType
ALU = mybir.AluOpType

RG = [list(range(N_CORES))]

# conv1 kernel positions, center (1,1) last so it carries stop=True
KPOS = [(0, 0), (0, 1), (0, 2), (1, 0), (1, 2), (2, 0), (2, 1), (2, 2), (1, 1)]


def _emit_conv1_pair(nc, ps_list, sx_list, w1t, ec, masks, h):
    """conv1 for a pair of images (shared weight loads), half h.

    sx tiles are [128, 56, 2, 28] (row, parity, col) {0,1}-valued f16;
    weights are 2*sign(w).  psum layout: rows 0-13 at bank0 cols 0-391,
    rows 14-27 at bank1 (offset 512).
    """
    # edge correction first: writes every pixel -> start=True
    for ps in ps_list:
        for b in range(2):
            nc.tensor.matmul(
                ps[:, 512 * b:512 * b + 392],
                ec[:, 128 * h:128 * h + 128],
                masks[:, 392 * b:392 * b + 392],
                start=True, stop=False)
    for kh, kw in KPOS:
        i0 = 1 if kh == 0 else 0
        j0 = 1 if kw == 0 else 0
        ncol = 28 - j0
        parity = 0 if kw == 1 else 1
        coff = 0  # odd-col index offset is 0 for both kw=0 and kw=2
        woff = ((kh * 3 + kw) * 2 + h) * 128
        last = (kh == 1 and kw == 1)
        for ps, sx in zip(ps_list, sx_list):
            for b in range(2):
                r0 = max(i0, 14 * b)
                nr = 14 * b + 14 - r0
                a0 = 2 * r0 + kh - 1
                bank = ps[:, 512 * b:512 * b + 392].rearrange(
                    "p (r c) -> p r c", c=28)
                out_v = bank[:, r0 - 14 * b:r0 - 14 * b + nr, j0:28]
                rhs = sx[:, a0:a0 + 2 * (nr - 1) + 1:2, parity,
                         coff:coff + ncol]
                nc.tensor.matmul(
                    out_v, w1t[:, woff:woff + 128], rhs,
                    start=False, stop=last)


def _affine_from_sq(nc, vp, gpk, g_ap, b_ap, s_out, b_out, thr_out, tag):
    """BN affine from packed global [128, 2(h), 2(sum,sumsq)] moments.

    s_out = g/sqrt(var+eps); b_out = b - mean*s; thr_out = -b_out.
    """
    mg = vp.tile([128, 2], F32, name="mg", tag=f"mg{tag}")
    nc.vector.tensor_scalar(mg[:], gpk[:, :, 0], 1.0 / NGLB, None, ALU.mult)
    e2 = vp.tile([128, 2], F32, name="e2", tag=f"e2{tag}")
    nc.vector.tensor_scalar(e2[:], gpk[:, :, 1], 1.0 / NGLB, None, ALU.mult)
    vr = vp.tile([128, 2], F32, name="vr", tag=f"vr{tag}")
    nc.vector.tensor_mul(vr[:], mg[:], mg[:])
    nc.vector.tensor_sub(vr[:], e2[:], vr[:])
    nc.vector.tensor_scalar(vr[:], vr[:], EPS, None, ALU.add)
    sd = vp.tile([128, 2], F32, name="sd", tag=f"sd{tag}")
    nc.scalar.sqrt(sd[:], vr[:])
    inv = vp.tile([128, 2], F32, name="inv", tag=f"inv{tag}")
    nc.vector.reciprocal(inv[:], sd[:])
    nc.vector.tensor_mul(s_out, inv[:], g_ap)
    t2 = vp.tile([128, 2], F32, name="t2", tag=f"t2{tag}")
    nc.vector.tensor_mul(t2[:], mg[:], s_out)
    nc.vector.tensor_sub(b_out, b_ap, t2[:])
    nc.vector.tensor_sub(thr_out, t2[:], b_ap)


def _inject_waits(nc, injections):
    """Append sem-ge waits to named instructions after Tile scheduling.

    Tile's single-core scheduling sim cannot model semaphore increments
    arriving from peer cores' remote DMAs, so those waits are attached
    to the already-scheduled program (the carrying instructions are in
    the local data-dependency chain, which pins their engine order).
    """
    by_name = {}
    for fn in nc.m.functions:
        for blk in fn.blocks:
            for ins in blk.instructions:
                by_name[ins.name] = ins
    for name, sem, val in injections:
        ins = by_name[name]
        w = mybir.SyncWait(
            sync_type="semaphore", id=sem.num, ant_name=sem.name,
            wait_mode="sem-ge-imm", wait_value=val)
        if ins.sync_info is None:
            ins.sync_info = mybir.SyncInfo(on_wait=[w], on_update=[])
        else:
            ins.sync_info.on_wait.append(w)


def _build():
    nc = bacc.Bacc(
        "TRN2",
        target_bir_lowering=False,
        debug=False,
        enable_asserts=False,
        num_devices=N_CORES,
    )
    xs = nc.dram_tensor("xs", [BL, C, H * W], F16, kind="ExternalInput")
    w1b = nc.dram_tensor("w1b", [128, 2304], F16, kind="ExternalInput")
    w2b = nc.dram_tensor("w2b", [128, 512], F16, kind="ExternalInput")
    ecb = nc.dram_tensor("ecb", [3, 256], F16, kind="ExternalInput")
    mkb = nc.dram_tensor("mkb", [3, 784], F16, kind="ExternalInput")
    vecs_d = nc.dram_tensor("vecs", [128, 8], F32, kind="ExternalInput")
    out_d = nc.dram_tensor("out", [BL, C, NPIX], F16, kind="ExternalOutput")

    xs_ap = xs.ap()
    out_ap = out_d.ap()

    if RDMA:
        rsem = nc.alloc_semaphore("ar_rsem")
        lsem = nc.alloc_semaphore("ar_lsem")
        psem = nc.alloc_semaphore("ar_psem")
        injections = []

    with tile.TileContext(nc) as tc:
        with tc.tile_pool(name="wp", bufs=1) as wp, \
             tc.tile_pool(name="xp", bufs=4) as xp, \
             tc.tile_pool(name="sxp", bufs=4) as sxp, \
             tc.tile_pool(name="tp", bufs=8) as tp, \
             tc.tile_pool(name="sqp", bufs=2) as sqp, \
             tc.tile_pool(name="vp", bufs=2) as vp, \
             tc.tile_pool(name="pp", bufs=4, space="PSUM") as pp, \
             tc.tile_pool(name="dramp", bufs=1, space="DRAM") as dramp:

            wmt = wp.tile([128, 1], F32)
            nc.vector.memset(wmt[:], 0.0)

            # ---- warmup collective: the ncfw path's one-time init barrier
            # and first-op cost run concurrently with phases A-C so the real
            # stage-2 AllReduce executes warm.  No critical section: that
            # would hold the sync engine (and the input stream) hostage.
            warm_in = dramp.tile([128, 1], F32)
            warm_out = dramp.tile([128, 1], F32)
            nc.scalar.dma_start(warm_in[:], wmt[:])
            nc.gpsimd.collective_compute(
                "AllReduce", ALU.add, replica_groups=RG,
                ins=[warm_in.opt()], outs=[warm_out.opt()])

            if RDMA:
                nc.gpsimd.load_library(library_config.remote_dma)
                send1 = wp.tile([128, 4], F32)
                gbuf1 = wp.tile([128, 8, 4], F32)
                send2 = wp.tile([128, 4], F32)
                gbuf2 = wp.tile([128, 8, 4], F32)

                def emit_preps(gbuf, send, gate_lsem=None):
                    for j in range(1, 8):
                        # D2D lanes (bit2) deliver to dest^2 (hw lane perm)
                        dj = j ^ 2 if j & 4 else j
                        rd = [(0, dj) if k == j else None for k in range(8)]
                        bi = nc.gpsimd.remote_dma_broadcast(
                            gbuf[:, j], send[:], rsem, lsem, rdests=rd)
                        if gate_lsem and j == 1:
                            # don't desc-gen while the ring still holds the
                            # previous exchange's frames
                            injections.append((bi.ins.name, lsem, gate_lsem))

                emit_preps(gbuf1, send1)


            # ---- constants ----
            w1t = wp.tile([128, 2304], F16)
            nc.scalar.dma_start(w1t[:], w1b.ap())
            w2t = wp.tile([128, 512], F16)
            nc.scalar.dma_start(w2t[:], w2b.ap())
            ec = wp.tile([3, 256], F16)
            nc.scalar.dma_start(ec[:], ecb.ap())
            masks = wp.tile([3, 784], F16)
            nc.scalar.dma_start(masks[:], mkb.ap())
            vecs = wp.tile([128, 8], F32)
            nc.scalar.dma_start(vecs[:], vecs_d.ap())

            # warm the ACT table sets (sqrt is on the AR critical path)
            wsq = wp.tile([128, 1], F32)
            nc.scalar.sqrt(wsq[:], wmt[:])
            nc.scalar.square(wsq[:], wmt[:])

            # persistent big tiles
            c1t = wp.tile([128, 2, BL, NPIX], F16)     # conv1 (h, n)
            mbt = wp.tile([128, 2, BL, NPIX], F16)     # maxpool (h, n)
            x1t = wp.tile([128, 2, BL, NPIX], F16)     # x1 = s1*c1 + m
            sx1t = wp.tile([128, 2, BL, NPIX], F16)    # q2 (h, n)
            c2t = wp.tile([128, 2, BL, NPIX], F16)     # conv2 (co, n)
            otb = wp.tile([128, 2, BL, NPIX], F16)     # output (co, n)
            st1 = wp.tile([128, 2, 2, BL], F32)        # [h, kind, n]
            st2 = wp.tile([128, 2, 2, BL], F32)        # [co, kind, n]
            pk1 = wp.tile([128, 2, 2], F32)
            pk2 = wp.tile([128, 2, 2], F32)
            g1pk = wp.tile([128, 2, 2], F32)
            g2pk = wp.tile([128, 2, 2], F32)
            s1v = wp.tile([128, 2], F32)
            b1v = wp.tile([128, 2], F32)
            thr1 = wp.tile([128, 2], F32)
            s2v = wp.tile([128, 2], F32)
            b2v = wp.tile([128, 2], F32)
            thr2d = wp.tile([128, 2], F32)
            bbv = wp.tile([128, 2], F32)

            sx_t = {}

            # ======== phase A: load, q1, conv1, moments; maxpool deferred
            def stage_a_load(h, n):
                xt = xp.tile([128, H * W], F16, name="xt")
                nc.sync.dma_start(xt[:], xs_ap[n, 128 * h:128 * h + 128])
                sx = sxp.tile([128, H, 2, OW], F16, name="sx")
                nc.vector.tensor_scalar(
                    sx.rearrange("p a b c -> p (a b c)"), xt[:],
                    0.0, None, ALU.is_ge)
                sx_t[(h, n)] = (xt, sx)

            def stage_a_stats(h, n, ps):
                pv = ps.rearrange("p (b x) -> p b x", x=512)[:, :, 0:392]
                cv = c1t[:, h, n].rearrange("p (b x) -> p b x", b=2)
                nc.scalar.activation(
                    cv, pv, AF.Copy, accum_out=st1[:, h, 0, n:n + 1])
                sq = sqp.tile([128, 2, 392], BF16, name="sq")
                nc.scalar.activation(
                    sq[:], pv, AF.Square, accum_out=st1[:, h, 1, n:n + 1])

            t_t = {}

            def stage_a_vpool(h, n):
                # vertical maxpool stage: the last reader of xt
                xt, _ = sx_t[(h, n)]
                xv = xt.rearrange("p (a b c) -> p a b c", a=H, b=2)
                t = tp.tile([128, OH, 2, OW], F16, name="t")
                nc.vector.tensor_max(t[:], xv[:, 0:56:2], xv[:, 1:56:2])
                nc.vector.tensor_max(t[:, 1:28], t[:, 1:28], xv[:, 1:54:2])
                t_t[(h, n)] = t

            def stage_a_hpool(h, n):
                # horizontal stage, deferred into the AllReduce window
                t = t_t[(h, n)]
                m = mbt[:, h, n].rearrange("p (a b) -> p a b", a=OH)
                nc.vector.tensor_max(m[:, :, :], t[:, :, 0], t[:, :, 1])
                nc.vector.tensor_max(m[:, :, 1:28], m[:, :, 1:28],
                                     t[:, :, 1, 0:27])

            def _pack(st, dest, base):
                for a in range(2):
                    for k in range(2):
                        u = vp.tile([128, 2], F32, name="u",
                                    tag=f"u{base}{a}{k}")
                        nc.vector.tensor_add(
                            u[:], st[:, a, k, 0:2], st[:, a, k, 2:4])
                        nc.vector.tensor_add(
                            dest[:, 2 * a + k:2 * a + k + 1],
                            u[:, 0:1], u[:, 1:2])

            def _reduce_gather(gbuf, send, gpk, rtarget, tag):
                cp0 = nc.vector.tensor_copy(gbuf[:, 0], send[:])
                r1 = vp.tile([128, 4, 4], F32, name="r1", tag=f"r1{tag}")
                nc.vector.tensor_add(r1[:], gbuf[:, 0:4], gbuf[:, 4:8])
                r2 = vp.tile([128, 2, 4], F32, name="r2", tag=f"r2{tag}")
                nc.vector.tensor_add(r2[:], r1[:, 0:2], r1[:, 2:4])
                nc.vector.tensor_add(
                    gpk.rearrange("p a b -> p (a b)"), r2[:, 0], r2[:, 1])

            def ar1_send():
                with tc.high_priority():
                    _pack(st1, send1, "s1")
                    nc.vector.sem_inc(psem, 1)
                    nc.gpsimd.wait_ge(psem, 1)
                    nc.gpsimd.trigger_dma(count=7)
                if RDMA_AR2:
                    # AR2's desc-gen follows on the gpsimd FIFO (runs
                    # during phase C), gated on AR1's ring retirement
                    emit_preps(gbuf2, send2, gate_lsem=112)

            def ar1_fin():
                # marker op reading the last hpool output: pins the DVE
                # queue so the blocking rsem wait sits AFTER the maxpool
                # work (which fills the exchange-latency window)
                mk = nc.vector.tensor_copy(
                    gbuf1[:, 0, 0:1], mbt[:, 1, BL - 1, 0:1])
                injections.append((mk.ins.name, rsem, 14))
                _reduce_gather(gbuf1, send1, g1pk, 14, "g1")
                _affine_from_sq(
                    nc, vp, g1pk, vecs[:, 0:2], vecs[:, 2:4],
                    s1v[:], b1v[:], thr1[:], tag="a1")

            def ar1():
              with tc.high_priority():
                _pack(st1, pk1.rearrange("p a b -> p (a b)"), "s1")
                ain = dramp.tile([128, 4], F32, name="ar1in")
                aout = dramp.tile([128, 4], F32, name="ar1out")
                nc.sync.dma_start(ain[:], pk1.rearrange("p a b -> p (a b)"))
                nc.gpsimd.collective_compute(
                    "AllReduce", ALU.add, replica_groups=RG,
                    ins=[ain.opt()], outs=[aout.opt()])
                nc.sync.dma_start(g1pk.rearrange("p a b -> p (a b)"), aout[:])
                _affine_from_sq(
                    nc, vp, g1pk, vecs[:, 0:2], vecs[:, 2:4],
                    s1v[:], b1v[:], thr1[:], tag="a1")

            for h in range(2):
                for n in range(BL):
                    stage_a_load(h, n)
                for w in range(2):
                    ns = (2 * w, 2 * w + 1)
                    ps_list = [pp.tile([128, 1024], F32, name="ps",
                                       tag="ps") for _ in ns]
                    _emit_conv1_pair(
                        nc, ps_list,
                        [sx_t[(h, n)][1] for n in ns], w1t, ec, masks, h)
                    for ps, n in zip(ps_list, ns):
                        stage_a_stats(h, n, ps)
                        stage_a_vpool(h, n)
            if RDMA:
                ar1_send()
            else:
                ar1()
            # horizontal maxpool overlaps the exchange latency
            for h in range(2):
                for n in range(BL):
                    stage_a_hpool(h, n)
            if RDMA:
                ar1_fin()

            # ======== phase C: x1, q2, conv2, moments ========
            def stage_c1(h, n):
                mfl = mbt[:, h, n]
                nc.vector.scalar_tensor_tensor(
                    x1t[:, h, n], c1t[:, h, n], s1v[:, h:h + 1], mfl,
                    ALU.mult, ALU.add)
                nc.vector.tensor_scalar(
                    sx1t[:, h, n], x1t[:, h, n], thr1[:, h:h + 1], None,
                    ALU.is_ge)

            def stage_c2(co, n):
                ps2 = pp.tile([128, 1024], F32, name="ps2", tag="ps")
                for ci in range(2):
                    woff = (ci * 2 + co) * 128
                    for cc0, ccn in ((0, 512), (512, NPIX - 512)):
                        nc.tensor.matmul(
                            ps2[:, cc0:cc0 + ccn],
                            w2t[:, woff:woff + 128],
                            sx1t[:, ci, n][:, cc0:cc0 + ccn],
                            start=(ci == 0), stop=(ci == 1))
                cv = c2t[:, co, n]
                nc.scalar.activation(
                    cv, ps2[:, 0:NPIX], AF.Copy,
                    accum_out=st2[:, co, 0, n:n + 1])
                # sumsq on the DVE (f16 2x) from the evicted copy; frees
                # the Scalar engine, which otherwise tails phase C
                sq = sqp.tile([128, 2, 392], BF16, name="sq")
                nc.vector.scalar_tensor_tensor(
                    sq.rearrange("p a b -> p (a b)"), cv, 1.0, cv,
                    ALU.mult, ALU.mult,
                    accum_out=st2[:, co, 1, n:n + 1])

            def ar2():
                if RDMA and RDMA_AR2:
                    _pack(st2, send2, "s2")
                    nc.vector.sem_inc(psem, 1)
                    nc.gpsimd.wait_ge(psem, 2)
                    nc.gpsimd.trigger_dma(count=7)
                    mk2 = nc.vector.tensor_copy(
                        gbuf2[:, 0, 0:1], send2[:, 0:1])
                    injections.append((mk2.ins.name, rsem, 28))
                    _reduce_gather(gbuf2, send2, g2pk, 28, "g2")
                    _affine_from_sq(
                        nc, vp, g2pk, vecs[:, 4:6], vecs[:, 6:8],
                        s2v[:], b2v[:], thr2d[:], tag="a2")
                    nc.vector.tensor_add(bbv[:], b1v[:], b2v[:])
                    return
                with tc.high_priority():
                    _pack(st2, pk2.rearrange("p a b -> p (a b)"), "s2")
                    ain = dramp.tile([128, 4], F32, name="ar2in")
                    aout = dramp.tile([128, 4], F32, name="ar2out")
                    nc.sync.dma_start(
                        ain[:], pk2.rearrange("p a b -> p (a b)"))
                    nc.gpsimd.collective_compute(
                        "AllReduce", ALU.add, replica_groups=RG,
                        ins=[ain.opt()], outs=[aout.opt()])
                    nc.sync.dma_start(
                        g2pk.rearrange("p a b -> p (a b)"), aout[:])
                    _affine_from_sq(
                        nc, vp, g2pk, vecs[:, 4:6], vecs[:, 6:8],
                        s2v[:], b2v[:], thr2d[:], tag="a2")
                    nc.vector.tensor_add(bbv[:], b1v[:], b2v[:])

            for n in range(BL):
                stage_c1(0, n)
                stage_c1(1, n)
            for co in range(2):
                for n in range(BL):
                    stage_c2(co, n)
            ar2()

            # ======== phase E: out = (s2*c2 + b1v+b2v) + x1 ========
            for co in range(2):
                for n in range(BL):
                    nc.vector.tensor_scalar(
                        otb[:, co, n], c2t[:, co, n],
                        s2v[:, co:co + 1], bbv[:, co:co + 1],
                        ALU.mult, ALU.add)
                    nc.vector.tensor_add(
                        otb[:, co, n], otb[:, co, n], x1t[:, co, n])
                nc.sync.dma_start(
                    out_ap[:, 128 * co:128 * co + 128].rearrange(
                        "n p x -> p n x"),
                    otb[:, co])

    if RDMA:
        _inject_waits(nc, injections)
    nc.compile()
    return nc


_NC = None


def _get_nc():
    global _NC
    if _NC is None:
        _NC = _build()
    return _NC


def _prep_inputs(x, w1, g1, b1, w2, g2, b2):
    """Host-side dtype/layout prep (weights tiny; x cast+deinterleave)."""
    x = np.asarray(x, dtype=np.float32)
    # column de-interleave: [B, C, H, 28, 2] -> [B, C, H, 2, 28]
    xr = x.reshape(B, C, H, OW, 2).transpose(0, 1, 2, 4, 3)
    x16 = np.ascontiguousarray(xr.reshape(B, C, H * W)).astype(np.float16)

    sw1 = np.sign(w1.astype(np.float32))            # [256, 64, 3, 3]
    t1 = np.zeros((128, 3, 3, 2, 128), np.float32)  # [ci_l, kh, kw, h, co_l]
    for h in range(2):
        for bb in range(2):
            blk = sw1[128 * h + 64 * bb:128 * h + 64 * bb + 64]
            t1[64 * bb:64 * bb + 64, :, :, h, 64 * bb:64 * bb + 64] = \
                2.0 * blk.transpose(1, 2, 3, 0)
    w1bv = t1.reshape(128, 2304).astype(np.float16)

    colsum1 = sw1.sum(axis=1)                       # [256, 3, 3]
    ecw = np.stack([
        colsum1[:, 0, :].sum(-1),                   # rowsum_top
        colsum1[:, :, 0].sum(-1),                   # colsum_left
        -colsum1[:, 0, 0],                          # -c00
    ]).astype(np.float16)                           # [3, 256]

    mk = np.zeros((3, OH, OW), np.float32)
    mk[0, 0, :] = 1.0
    mk[1, :, 0] = 1.0
    mk[2, 0, 0] = 1.0
    mkv = mk.reshape(3, NPIX).astype(np.float16)

    sw2 = np.sign(w2.astype(np.float32)[:, :, 0, 0])  # [256 co, 256 ci]
    t2 = np.zeros((128, 2, 2, 128), np.float32)       # [ci_l, ci, co, co_l]
    for ci in range(2):
        for co in range(2):
            t2[:, ci, co, :] = 2.0 * sw2[128 * co:128 * co + 128,
                                         128 * ci:128 * ci + 128].T
    w2bv = t2.reshape(128, 512).astype(np.float16)

    vecs = np.zeros((128, 8), np.float32)
    vecs[:, 0] = g1[:128]
    vecs[:, 1] = g1[128:]
    vecs[:, 2] = b1[:128]
    vecs[:, 3] = b1[128:]
    vecs[:, 4] = g2[:128]
    vecs[:, 5] = g2[128:]
    vecs[:, 6] = b2[:128]
    vecs[:, 7] = b2[128:]

    in_maps = []
    for i in range(N_CORES):
        in_maps.append({
            "xs": np.ascontiguousarray(x16[BL * i:BL * (i + 1)]),
            "w1b": w1bv,
            "w2b": w2bv,
            "ecb": ecw,
            "mkb": mkv,
            "vecs": vecs,
        })
    return in_maps


def run(x, w1, g1, b1, w2, g2, b2, trace=False):
    nc = _get_nc()
    in_maps = _prep_inputs(x, w1, g1, b1, w2, g2, b2)
    res = bass_utils.run_bass_kernel_spmd(
        nc, in_maps, core_ids=list(range(N_CORES)), trace=trace)
    out = np.concatenate(
        [res.results[i]["out"] for i in range(N_CORES)], axis=0)
    out = out.reshape(B, C, OH, OW).astype(np.float32)
    return out, res


def kernel(**inputs):
    out, _ = run(
        inputs["x"], inputs["w1"], inputs["g1"], inputs["b1"],
        inputs["w2"], inputs["g2"], inputs["b2"])
    return out
